# revision 10
# baseline (speedup 1.0000x reference)
"""Trainium2 Bass kernel for nn_MemoryGraphBackprop (GNN message passing).

Strategy
--------
T=64 sequential steps over state [BS=2, N=1024, D=64].  The recurrence is
PE-bound: the dense adjacency (A = 1024x1024 bf16) must stream through the
PE array every step (8192 rows = 3.41us at 2.4GHz), and an 8-core shard
would need a per-step HBM-bounce collective (>=15us floor) that dwarfs the
compute.  So: ONE NeuronCore, fully SBUF-resident recurrence.

Math (per step t):
    r   = A @ pm  (+ cc_t into nodes < C)
    dt  = decay * (1 - eot[b,t])
    h'  = dt*h + (1-dt)*r
    pm' = tanh(prim * h')
With u := prim*h the update is u' = dt*u + (1-dt)*prim*r (+ inject), which
maps to TWO fused scalar_tensor_tensor ops per quarter on DVE when decay is
spatially uniform (the spec's fill):  y = (ps * (1-dt)) * prim ;
u' = (u * dt) + y.  The cc inject is fully host-precomputed per call
(ccw_t = (1-dt_t)*prim[:, :C]*cc_t, since eot is a host-visible input) and
lands as one small tensor_add.

Schedule (the v2 redesign): the step's matmuls are split into THREE psum
groups over n: [0:512), [512:768), [768:1024).  The small late groups
produce the last pm chunks (4..7 = next step's late lhsT chunks), so each
group's chain(DVE) -> transpose(PE) -> tanh(ACT) tail hides under the
following matmul stream and the PE stays content-bound (24 MMs = 8192 rows
+ 8 transposes ~ 3.9us/step).  The fp32 output slice is emitted on ACT
AFTER tanh q0/q1 so it never delays the critical late-chunk tanhs.

Layouts:
  l2 (state u, psum r):  [128 part = b*64+d, 1024 free = n]
  l1 (pm, matmul lhsT):  [128 part = n%128, free = (n//128)*128 + b*64 + d]
"""

import sys

if "/opt/trn_rl_repo" not in sys.path:
    sys.path.insert(0, "/opt/trn_rl_repo")

import numpy as np

import concourse.bass as bass
import concourse.mybir as mybir
import concourse.tile as tile
from concourse import bass_utils

BS, T, C, D = 2, 64, 64, 64
N = 1024
NT = N // 128  # 8 node chunks
P = 128        # BS*D partitions in layout-2
NQ = 4         # chain quarters
QW = N // NQ   # 256

F32 = mybir.dt.float32
BF16 = mybir.dt.bfloat16

# n-ranges of the three psum matmul groups
GRP = [(0, 512), (512, 768), (768, 1024)]
# quarter -> (group index, offset within group)
QMAP = [(0, 0), (0, 256), (1, 0), (2, 0)]

# ---------------------------------------------------------------------------
# Workaround: this container's walrus accepts only ONE sync-wait per
# instruction.  (1) Tile's tail drain attaches one wait per live semaphore —
# split across multiple drains.  (2) Any multi-wait instruction gets its
# extra waits hoisted onto InstEventSemaphore carriers just before it.
# ---------------------------------------------------------------------------
from concourse.vector_clock import ScopedClock  # noqa: E402


def _patched_drain_and_barrier(self, tick_clock, wait_clock):
    drain_inst = self.nc.sync.drain()
    wait_clock.add_sem_waits(
        drain_inst.ins, ScopedClock({None: tick_clock.global_clock})
    )
    si = drain_inst.ins.sync_info
    if si is not None and si.on_wait is not None and len(si.on_wait) > 1:
        waits = list(si.on_wait)
        drain_inst.ins.sync_info = mybir.SyncInfo(
            on_wait=[waits[0]], on_update=si.on_update
        )
        for w in waits[1:]:
            d2 = self.nc.sync.drain()
            d2.ins.sync_info = mybir.SyncInfo(on_wait=[w], on_update=[])

    self.nc.all_engine_barrier()
    assert self.sems is not None
    popped = self.nc._tile_sem_poison_stack.pop()
    assert popped is self._sem_poison
    self.nc.clear_and_free_semaphores(list(self.sems.allocated().values()))
    self.nc.all_engine_barrier()


tile.TileContext._drain_and_barrier = _patched_drain_and_barrier


def _split_multi_waits(nc):
    n_carriers = 0
    for bb in nc.m.functions[0].blocks:
        insts = list(bb.instructions)
        out = []
        changed = False
        for inst in insts:
            si = inst.sync_info
            if si is not None and si.on_wait is not None and len(si.on_wait) > 1:
                waits = list(si.on_wait)
                for w in waits[:-1]:
                    n_carriers += 1
                    carrier = mybir.InstEventSemaphore(
                        name=f"waitsplit-{n_carriers}", ins=[], outs=[]
                    )
                    carrier.engine = inst.engine
                    carrier.sync_info = mybir.SyncInfo(on_wait=[w], on_update=[])
                    out.append(carrier)
                inst.sync_info = mybir.SyncInfo(
                    on_wait=[waits[-1]], on_update=si.on_update
                )
                changed = True
            out.append(inst)
        if changed:
            bb.instructions = out
    return n_carriers


# ---------------------------------------------------------------------------
# Host-side input massaging (layouts, scatter into dense A, norms, sigmoid).
# ---------------------------------------------------------------------------
def _prep_host(inputs):
    import ml_dtypes

    bf16 = ml_dtypes.bfloat16

    cc = np.asarray(inputs["cc_signals"], dtype=np.float32)       # [B,T,C,D]
    eot = np.asarray(inputs["eot_mask"]).astype(bool)             # [B,T]
    idx = np.asarray(inputs["conn_indices"]).astype(np.int64)     # [N,K]
    cmask = np.asarray(inputs["conn_mask"]).astype(np.float32)    # [N,K]
    prim = np.asarray(inputs["primitives"], dtype=np.float32)     # [N,D]
    w = np.asarray(inputs["conn_weights"], dtype=np.float32)      # [N,K]
    dlog = np.asarray(inputs["decay_logit"], dtype=np.float32)    # [N]
    h0 = np.asarray(inputs["h0"], dtype=np.float32)               # [B,N,D]
    pm0 = np.asarray(inputs["prev_msg0"], dtype=np.float32)       # [B,N,D]

    # dense adjacency, transposed for the layout-2 matmul (rhs[m, n] = A[n, m])
    A = np.zeros((N, N), dtype=np.float32)
    np.add.at(A, (np.arange(N)[:, None], idx), w * cmask)
    At = np.ascontiguousarray(A.T)                                # [m, n]
    at_host = At.reshape(NT, 128, N).transpose(1, 0, 2).reshape(128, NT * N)

    # L2-normalized cc, layout-2: [b*64+d partitions, t*64 + n(<C) free]
    nrm = np.maximum(np.linalg.norm(cc, axis=-1, keepdims=True), 1e-8)
    ccn = (cc / nrm).astype(np.float32)
    cc2_host = np.ascontiguousarray(
        ccn.transpose(0, 3, 1, 2).reshape(P, T * C)
    )

    decay = (1.0 / (1.0 + np.exp(-dlog.astype(np.float64)))).astype(np.float32)
    uniform = bool(np.all(decay == decay[0]))

    prim_l2 = np.ascontiguousarray(np.tile(prim.T, (BS, 1)))      # [128, N]
    fmat = np.repeat((~eot).astype(np.float32), D, axis=0)        # [128, T]

    h0_l2 = h0.transpose(0, 2, 1).reshape(P, N)                   # [b*64+d, n]
    u0 = np.ascontiguousarray(prim_l2 * h0_l2)

    pm0_l1 = np.ascontiguousarray(
        pm0.reshape(BS, NT, 128, D).transpose(2, 1, 0, 3).reshape(128, NT * P)
    )

    host = {
        "at": at_host.astype(bf16),
        "prim": prim_l2.astype(bf16),
        "u0": u0.astype(bf16),
        "pm0": pm0_l1.astype(bf16),
    }
    if uniform:
        g = decay[0] * fmat                                       # [128, T]
        host["gmat"] = np.ascontiguousarray(g.astype(np.float32))
        host["h1g"] = np.ascontiguousarray((1.0 - g).astype(np.float32))
        # fully pre-weighted cc inject: ccw[:, t*C+j] = (1-dt[b,t])*prim[b*64+d, j]*cc
        h1g_rep = np.repeat(1.0 - g, C, axis=1) if False else None
        ccw = cc2_host.reshape(P, T, C) * (1.0 - g)[:, :, None] \
            * prim_l2[:, :C][:, None, :]
        host["ccw"] = np.ascontiguousarray(ccw.reshape(P, T * C).astype(bf16))
    else:
        host["cc2"] = cc2_host.astype(bf16)
        dec_l2 = np.ascontiguousarray(np.broadcast_to(decay[None, :], (P, N)))
        host["dec"] = dec_l2.astype(bf16)
        host["dp"] = (prim_l2 * decay[None, :]).astype(bf16)
        host["fmat"] = np.ascontiguousarray(fmat.astype(np.float32))
    return host, uniform


# ---------------------------------------------------------------------------
# Device kernel
# ---------------------------------------------------------------------------
def _build_bass(uniform):
    nc = bass.Bass("TRN2", target_bir_lowering=False, debug=False)

    at_d = nc.dram_tensor("at", [128, NT * N], BF16, kind="ExternalInput")
    prim_d = nc.dram_tensor("prim", [P, N], BF16, kind="ExternalInput")
    u0_d = nc.dram_tensor("u0", [P, N], BF16, kind="ExternalInput")
    pm0_d = nc.dram_tensor("pm0", [128, NT * P], BF16, kind="ExternalInput")
    out_d = nc.dram_tensor("out", [T, C, P], F32, kind="ExternalOutput")
    if uniform:
        g_d = nc.dram_tensor("gmat", [P, T], F32, kind="ExternalInput")
        h1g_d = nc.dram_tensor("h1g", [P, T], F32, kind="ExternalInput")
        ccw_d = nc.dram_tensor("ccw", [P, T * C], BF16, kind="ExternalInput")
    else:
        cc2_d = nc.dram_tensor("cc2", [P, T * C], BF16, kind="ExternalInput")
        dec_d = nc.dram_tensor("dec", [P, N], BF16, kind="ExternalInput")
        dp_d = nc.dram_tensor("dp", [P, N], BF16, kind="ExternalInput")
        f_d = nc.dram_tensor("fmat", [P, T], F32, kind="ExternalInput")

    Tanh = mybir.ActivationFunctionType.Tanh
    MUL = mybir.AluOpType.mult
    ADD = mybir.AluOpType.add

    with tile.TileContext(nc) as tc:
        with (
            tc.tile_pool(name="consts", bufs=1) as consts,
            tc.tile_pool(name="state", bufs=3) as state,
            tc.tile_pool(name="tmp", bufs=3) as tmp,
            tc.tile_pool(name="pg0", bufs=2, space="PSUM") as pg0,
            tc.tile_pool(name="pg12", bufs=2, space="PSUM") as pg12,
            tc.tile_pool(name="ptp", bufs=2, space="PSUM") as ptp,
        ):
            # --- load state + small constants first so step-0 deps clear
            # --- early, then the big A slab ---
            id128_sb = consts.tile([128, 128], BF16)
            from concourse.masks import make_identity
            make_identity(nc, id128_sb[:])

            # HAM warm-up: dummy matmuls on the identity keep the PE
            # activity monitor at full clock while the input DMAs land.
            warm_ps = pg0.tile([128, 128], F32, tag="g0", name="warm_ps")
            for i in range(64):
                nc.tensor.matmul(
                    warm_ps[:], id128_sb[:], id128_sb[:],
                    start=(i == 0), stop=(i == 63), skip_group_check=True,
                )

            u = state.tile([P, N], BF16, tag="u", name="u_init")
            pm = state.tile([128, N], BF16, tag="pm", name="pm_init")
            for h in range(2):
                sl = slice(h * 512, (h + 1) * 512)
                nc.sync.dma_start(out=u[:, sl], in_=u0_d.ap()[:, sl])
                nc.sync.dma_start(out=pm[:, sl], in_=pm0_d.ap()[:, sl])
            at_sb = consts.tile([128, NT * N], BF16)
            for h in range(2):
                for m in range(NT):
                    sl = slice(m * N + h * 512, m * N + (h + 1) * 512)
                    nc.sync.dma_start(out=at_sb[:, sl], in_=at_d.ap()[:, sl])
            prim_sb = consts.tile([P, N], BF16)
            nc.sync.dma_start(out=prim_sb[:], in_=prim_d.ap()[:])
            if uniform:
                g_sb = consts.tile([P, T], F32)
                nc.sync.dma_start(out=g_sb[:], in_=g_d.ap()[:])
                h1g_sb = consts.tile([P, T], F32)
                nc.sync.dma_start(out=h1g_sb[:], in_=h1g_d.ap()[:])
                ccw_sb = consts.tile([P, T * C], BF16)
                for q in range(4):
                    s = slice(q * (T * C) // 4, (q + 1) * (T * C) // 4)
                    nc.sync.dma_start(out=ccw_sb[:, s], in_=ccw_d.ap()[:, s])
            else:
                dec_sb = consts.tile([P, N], BF16)
                nc.sync.dma_start(out=dec_sb[:], in_=dec_d.ap()[:])
                dp_sb = consts.tile([P, N], BF16)
                nc.sync.dma_start(out=dp_sb[:], in_=dp_d.ap()[:])
                f_sb = consts.tile([P, T], F32)
                nc.sync.dma_start(out=f_sb[:], in_=f_d.ap()[:])
                cc2_sb = consts.tile([P, T * C], BF16)
                for q in range(4):
                    s = slice(q * (T * C) // 4, (q + 1) * (T * C) // 4)
                    nc.sync.dma_start(out=cc2_sb[:, s], in_=cc2_d.ap()[:, s])

            for t in range(T):
                last = (t == T - 1)
                # ---- matmuls: r = A @ pm into 3 psum groups over n ----
                ps_g0 = pg0.tile([P, 512], F32, tag="g0", name="ps_g0")
                ps_g12 = pg12.tile([P, 512], F32, tag="g12", name="ps_g12")
                # (tile, lo, hi): psum target ranges of the three groups
                psd = [(ps_g0, 0, 512), (ps_g12, 0, 256), (ps_g12, 256, 512)]
                un = state.tile([P, N], BF16, tag="u", name="un")
                pmn = state.tile([128, N], BF16, tag="pm", name="pmn")
                ptsall = ptp.tile([128, N], BF16, tag="pt", name="pts")
                # group-outer order: g0 (n 0:512) completes ~2/3 of a period
                # early, g1 next, g2 last — staggering each group's
                # chain->transpose->tanh tail under the remaining MM stream.
                for gi, (n0, n1) in enumerate(GRP):
                    pt, lo, hi = psd[gi]
                    for m in range(NT):
                        nc.tensor.matmul(
                            pt[:, lo:hi],
                            pm[:, m * 128:(m + 1) * 128],
                            at_sb[:, m * N + n0: m * N + n1],
                            start=(m == 0),
                            stop=(m == NT - 1),
                            skip_group_check=True,
                        )

                # ---- chain on DVE: 2 fused stt per quarter ----
                if not uniform:
                    ft = f_sb[:, t:t + 1]
                    w0 = tmp.tile([P, N], BF16, tag="w0")
                    nc.vector.tensor_scalar_mul(w0[:], dec_sb[:], ft)
                    w1 = tmp.tile([P, N], BF16, tag="w1")
                    nc.vector.tensor_scalar_mul(w1[:], dp_sb[:], ft)
                    w2 = tmp.tile([P, N], BF16, tag="w2")
                    nc.vector.tensor_sub(w2[:], prim_sb[:], w1[:])
                for q in range(NQ):
                    if last and q > 0:
                        continue  # last step: only chunk 0 reaches out
                    gi, off = QMAP[q]
                    pt, lo, _hi = psd[gi]
                    psl = pt[:, lo + off:lo + off + QW]
                    qsl = slice(q * QW, (q + 1) * QW)
                    if uniform:
                        y = tmp.tile([P, QW], BF16, tag=f"y{q}", name=f"y{q}")
                        nc.vector.scalar_tensor_tensor(
                            y[:], psl, h1g_sb[:, t:t + 1], prim_sb[:, qsl],
                            op0=MUL, op1=MUL,
                        )
                        nc.vector.scalar_tensor_tensor(
                            un[:, qsl], u[:, qsl], g_sb[:, t:t + 1], y[:],
                            op0=MUL, op1=ADD,
                        )
                        if q == 0:
                            nc.vector.tensor_add(
                                un[:, 0:C], un[:, 0:C],
                                ccw_sb[:, t * C:(t + 1) * C],
                            )
                    else:
                        sbq = tmp.tile([P, QW], BF16, tag=f"sb{q}")
                        nc.vector.tensor_mul(sbq[:], u[:, qsl], w0[:, qsl])
                        x = tmp.tile([P, QW], BF16, tag=f"x{q}")
                        nc.vector.tensor_mul(x[:], psl, w2[:, qsl])
                        nc.vector.tensor_add(un[:, qsl], x[:], sbq[:])
                        if q == 0:
                            cw = tmp.tile([P, C], BF16, tag="cw")
                            nc.vector.tensor_mul(
                                cw[:], w2[:, 0:C],
                                cc2_sb[:, t * C:(t + 1) * C],
                            )
                            nc.vector.tensor_add(
                                un[:, 0:C], un[:, 0:C], cw[:]
                            )

                # ---- transposes (PE) + tanh (ACT); out slice after q1 so it
                # ---- never delays the critical late-chunk tanhs ----
                out_sb = tmp.tile([C, P], F32, tag="out_sb")
                for q in range(NQ):
                    if last and q > 0:
                        continue
                    for j in range(2):
                        if last and j > 0:
                            continue
                        mloc = q * 2 + j
                        nc.tensor.transpose(
                            ptsall[:, mloc * 128:(mloc + 1) * 128],
                            un[:, mloc * 128:(mloc + 1) * 128],
                            id128_sb[:],
                        )
                    if not last:
                        nc.scalar.activation(
                            pmn[:, q * QW:(q + 1) * QW],
                            ptsall[:, q * QW:(q + 1) * QW], Tanh,
                        )
                    if q == (0 if last else 1):
                        nc.scalar.activation(out_sb[:], ptsall[0:C, 0:P], Tanh)
                        nc.sync.dma_start(out=out_d.ap()[t], in_=out_sb[:])

                u, pm = un, pmn

    _split_multi_waits(nc)
    return nc


RUN_KWARGS: dict = {}
_BUILT: dict = {}


def _get_built(uniform):
    if uniform not in _BUILT:
        _BUILT[uniform] = _build_bass(uniform)
    return _BUILT[uniform]


def kernel(**inputs) -> np.ndarray:
    host, uniform = _prep_host(inputs)
    nc = _get_built(uniform)
    res = bass_utils.run_bass_kernel_spmd(nc, [host], core_ids=[0], **RUN_KWARGS)
    kernel.last_result = res
    out_dev = res.results[0]["out"]                               # [T, C, 128]
    out = out_dev.reshape(T, C, BS, D).transpose(2, 0, 1, 3)      # [B,T,C,D]
    return np.ascontiguousarray(out)


if __name__ == "__main__":
    print("standalone smoke: building bass module (uniform decay path)...")
    _get_built(True)
    print("built ok")
